# revision 12
# baseline (speedup 1.0000x reference)
"""Trainium2 Bass kernel for nn_NodePooling (segment mean pooling).

Reference computation:
    features [500000, 8, 64] f32, counts [5000] i32 (uniform 100)
    x = features.transpose(0,2,1).reshape(N, 512)
    out[g] = mean over graph g's nodes of x  -> [5000, 512]

Strategy:
  - Host: view features as [N, 512] (contiguous); shard 5000 graphs as 625
    graphs (62500 rows) per core across 8 NeuronCores. The (P,D)->(D,P)
    permutation commutes with the segment mean, so the device reduces in
    natural (P,D) layout and the host permutes the tiny [5000, 512] result.
  - Device (per core): stream 20 big contiguous DMAs (19x [128 part x 12800]
    = 3200 rows = 32 graphs, plus a [68 x 12800] tail = 17 graphs). In this
    layout partition p holds 25 consecutive rows, so each graph (100 rows)
    is exactly 4 consecutive partitions.
    Reduction is split across engines to stay under the ~358 GB/s/core HBM
    DMA roofline (~360 us):
      * PE tiles: 25 accumulating fp32 matmuls (lhsT = 0/1 selection matrix
        S[128,32], S[p,g] = (p//4==g)) -> psum [32,512] graph sums.
      * DVE tiles: one tensor_reduce over the per-partition 25 rows
        -> [128,512] partials, then a single fp32 matmul by S -> psum.
    ACT copies psum->SBUF; outputs DMA out via SWDGE.
  - Host: concat per-core [625, 512] results, divide by counts, permute.
"""

import numpy as np

N_NODES = 500_000
PATH = 8
DIM = 64
N_GRAPHS = 5_000
COLS = PATH * DIM  # 512
N_CORES = 8
ROWS_PER_CORE = N_NODES // N_CORES      # 62500
GRAPHS_PER_CORE = N_GRAPHS // N_CORES   # 625
CNT = N_NODES // N_GRAPHS               # 100 rows per graph

R = 25                   # rows per partition
FULL_P = 128             # partitions in a full tile
TILE_ROWS = FULL_P * R   # 3200 rows per full tile
TILE_G = TILE_ROWS // CNT               # 32 graphs per full tile
N_FULL_TILES = 19
TAIL_ROWS = ROWS_PER_CORE - N_FULL_TILES * TILE_ROWS  # 1700
TAIL_P = TAIL_ROWS // R  # 68
TAIL_G = TAIL_ROWS // CNT               # 17
PE_EVERY = 3             # every 3rd full tile reduced on the PE (matmul) path
PS_BUFS = 4              # PSUM accumulator slots
OT_BUFS = 4              # output staging slots (must equal PS_BUFS: the
                         # absorber ldweights relies on ot[t-PS_BUFS] having
                         # been written by the ACT copy that freed ps slot t)

_CACHE = {}
LAST_RESULT = None  # BassKernelResults from the most recent run (for test.py)


def _build_nc():
    import concourse.bass as bass
    import concourse.mybir as mybir
    from concourse.tile import TileContext

    f32 = mybir.dt.float32
    nc = bass.Bass()

    x = nc.dram_tensor("x", [ROWS_PER_CORE, COLS], f32, kind="ExternalInput")
    y = nc.dram_tensor("y", [GRAPHS_PER_CORE, COLS], f32, kind="ExternalOutput")

    # 0/1 selection matrix: S[p, g] = 1 if partition p belongs to graph g.
    s_np = np.zeros((FULL_P, TILE_G), dtype=np.float32)
    for p in range(FULL_P):
        s_np[p, p // (CNT // R)] = 1.0
    s_dram = nc.inline_tensor(s_np, name="s_sel")

    n_dve_tiles = sum(
        1
        for t in range(N_FULL_TILES + 1)
        if t == N_FULL_TILES or t % PE_EVERY != 0
    )
    with TileContext(nc) as tc:
        with (
            tc.tile_pool(name="sconst", bufs=1) as spool,
            tc.tile_pool(name="xin", bufs=3) as xpool,
            tc.tile_pool(name="partials", bufs=n_dve_tiles) as ppool,
            tc.tile_pool(name="psum", bufs=PS_BUFS, space="PSUM") as cpool,
            tc.tile_pool(name="outbuf", bufs=OT_BUFS) as opool,
        ):
            s_sb = spool.tile([FULL_P, TILE_G], f32)
            nc.sync.dma_start(out=s_sb[:], in_=s_dram[:])

            # Sync-wait budget: each lowered ISA instruction carries very few
            # semaphore waits (the PE matmul/ldweights slots hold just one),
            # but Tile attaches every outstanding dependency to the first
            # instruction that touches a tile. fp32 matmuls are worst: they
            # self-load weights, so lhsT+rhs+psum deps all land on one
            # instruction. Keep every fp32 matmul at <=1 wait by absorbing
            # deps into bf16 LDWEIGHTS/matmul helpers (bf16 matmul emits a
            # separate LDWEIGHTS, spreading waits over two instructions; a
            # bare LDWEIGHTS has no Tile-tracked output, so it picks up only
            # the read dep we aim it at). The bf16 views are bitcasts of live
            # fp32 tiles — values are irrelevant, only the dependency counts.
            s_b16 = s_sb[:1, 0:1].bitcast(mybir.dt.bfloat16)  # [1, 2]
            nc.tensor.ldweights(s_b16)  # absorbs the S-matrix DMA wait

            ots = []
            for t in range(N_FULL_TILES + 1):
                tail = t == N_FULL_TILES
                P = TAIL_P if tail else FULL_P
                G = TAIL_G if tail else TILE_G
                r0 = t * TILE_ROWS
                g0 = t * TILE_G

                xt = xpool.tile([FULL_P, R * COLS], f32, tag="xt")
                src = x[r0 : r0 + P * R, :].rearrange("(p r) c -> p (r c)", p=P)
                # Single issuing engine: alternating SP/ACT leaves each engine
                # with stale vector clocks and piles >2 sync waits onto DMA
                # instructions, which the PSEUDO_DMA ISA slot can't encode.
                nc.sync.dma_start(out=xt[:P, :], in_=src)

                ps = cpool.tile([TILE_G, COLS], f32, tag="ps")
                if t >= PS_BUFS:
                    # ot slot (t % OT_BUFS) was written by the same ACT copy
                    # that last read ps slot (t % PS_BUFS) — reading it syncs
                    # PE's view of ACT to exactly the needed tick.
                    nc.tensor.ldweights(ots[t - PS_BUFS][:1, 0:1].bitcast(mybir.dt.bfloat16))
                    # PSUM WAW vs our own older accumulation group.
                    nc.tensor.matmul(
                        ps[:2, :2],
                        lhsT=s_b16,
                        rhs=s_b16,
                        start=True,
                        stop=True,
                        skip_group_check=True,
                    )
                if not tail and t % PE_EVERY == 0:
                    # PE path: accumulate 25 matmuls into PSUM.
                    for r in range(R):
                        nc.tensor.matmul(
                            ps[:G, :],
                            lhsT=s_sb[:P, :G],
                            rhs=xt[:P, r * COLS : (r + 1) * COLS],
                            start=(r == 0),
                            stop=(r == R - 1),
                        )
                else:
                    # DVE path: reduce 25 rows per partition, then one matmul.
                    # Single-use slots (bufs=N_DVE_TILES) so the reduce never
                    # carries WAW/read-release waits — only its input DMA.
                    pt = ppool.tile([FULL_P, COLS], f32, tag="pt")
                    nc.vector.tensor_reduce(
                        pt[:P, :],
                        xt[:P, :].rearrange("p (r c) -> p c r", r=R),
                        axis=mybir.AxisListType.X,
                        op=mybir.AluOpType.add,
                    )
                    nc.tensor.matmul(
                        ps[:G, :],
                        lhsT=s_sb[:P, :G],
                        rhs=pt[:P, :],
                        start=True,
                        stop=True,
                    )

                ot = opool.tile([TILE_G, COLS], f32, tag="ot")
                ots.append(ot)
                nc.scalar.copy(out=ot[:G, :], in_=ps[:G, :])
                nc.gpsimd.dma_start(out=y[g0 : g0 + G, :], in_=ot[:G, :])

    _split_excess_waits(nc)
    return nc


def _split_excess_waits(nc):
    """Walrus encodes at most one semaphore wait per compute/DMA instruction
    (setupSyncWait raises "Too many sync wait commands" otherwise), but Tile's
    semaphore assignment attaches every outstanding dependency to the first
    instruction touching a tile. Hoist all but the last wait of each
    multi-wait instruction into dedicated wait-only InstEventSemaphore
    instructions inserted just before it on the same engine — the sequencer
    blocks there instead, which is semantically identical.
    """
    import concourse.mybir as mybir

    skip = {
        "InstEventSemaphore",
        "InstCall",
        "InstUnconditionalBranch",
        "InstISA",
        "InstRegisterMove",
    }
    n_fix = 0
    for bb in nc.main_func.blocks:
        lst = bb.instructions
        i = 0
        while i < len(lst):
            ins = lst[i]
            si = ins.sync_info
            if (
                type(ins).__name__ not in skip
                and si is not None
                and len(si.on_wait) > 1
            ):
                waits = list(si.on_wait)
                for w in waits[:-1]:
                    ev = mybir.InstEventSemaphore(
                        name=f"W-split-{n_fix}", ins=[], outs=[]
                    )
                    n_fix += 1
                    ev.engine = ins.engine
                    ev.sync_info = mybir.SyncInfo(on_wait=[w], on_update=[])
                    lst.insert(i, ev)
                    i += 1
                ins.sync_info = mybir.SyncInfo(
                    on_wait=[waits[-1]], on_update=list(si.on_update)
                )
            i += 1
    return n_fix


def _numpy_fallback(features, counts):
    n = features.shape[0]
    g = counts.shape[0]
    x = np.transpose(features, (0, 2, 1)).reshape(n, -1)
    out = np.zeros((g, x.shape[1]), dtype=np.float32)
    idx = 0
    for i in range(g):
        c = int(counts[i])
        if c > 0:
            out[i] = x[idx : idx + c].sum(axis=0, dtype=np.float32)
        idx += c
    denom = np.maximum(counts, 1).astype(np.float32)[:, None]
    return (out / denom).astype(np.float32)


def kernel(features, counts, _trace=False, _trace_cores=None):
    global LAST_RESULT
    features = np.ascontiguousarray(np.asarray(features, dtype=np.float32))
    counts = np.asarray(counts, dtype=np.int32)

    if (
        features.shape != (N_NODES, PATH, DIM)
        or counts.shape != (N_GRAPHS,)
        or not np.all(counts == CNT)
    ):
        return _numpy_fallback(features, counts)

    from concourse.bass_utils import run_bass_kernel_spmd

    if "nc" not in _CACHE:
        _CACHE["nc"] = _build_nc()
    nc = _CACHE["nc"]

    x_flat = features.reshape(N_NODES, COLS)
    in_maps = [
        {"x": x_flat[c * ROWS_PER_CORE : (c + 1) * ROWS_PER_CORE]}
        for c in range(N_CORES)
    ]

    res = run_bass_kernel_spmd(
        nc,
        in_maps,
        core_ids=list(range(N_CORES)),
        trace=_trace,
        trace_cores=_trace_cores,
    )
    LAST_RESULT = res

    sums = np.concatenate([r["y"] for r in res.results], axis=0)  # [5000, 512] (p,d)
    denom = np.maximum(counts, 1).astype(np.float32)[:, None]
    means = sums / denom
    # (g, p, d) -> (g, d, p) to match reference layout
    out = means.reshape(N_GRAPHS, PATH, DIM).transpose(0, 2, 1).reshape(N_GRAPHS, COLS)
    return np.ascontiguousarray(out.astype(np.float32))


# revision 17
# speedup vs baseline: 326.2544x; 326.2544x over previous
"""Trainium2 Bass kernel for nn_NodePooling (segment mean pooling).

Reference computation:
    features [500000, 8, 64] f32, counts [5000] i32 (uniform 100)
    x = features.transpose(0,2,1).reshape(N, 512)
    out[g] = mean over graph g's nodes of x  -> [5000, 512]

Strategy:
  - Host: view features as [N, 512] (contiguous); shard 5000 graphs as 625
    graphs (62500 rows) per core across 8 NeuronCores. The (P,D)->(D,P)
    permutation commutes with the segment mean, so the device reduces in
    natural (P,D) layout and the host permutes the tiny [5000, 512] result.
  - Device (per core): stream 20 big contiguous DMAs (19x [128 part x 12800]
    = 3200 rows = 32 graphs, plus a [68 x 12800] tail = 17 graphs). In this
    layout partition p holds 25 consecutive rows, so each graph (100 rows)
    is exactly 4 consecutive partitions.
    Reduction is split across engines to stay under the ~358 GB/s/core HBM
    DMA roofline (~360 us):
      * PE tiles: 25 accumulating fp32 matmuls (lhsT = 0/1 selection matrix
        S[128,32], S[p,g] = (p//4==g)) -> psum [32,512] graph sums.
      * DVE tiles: one tensor_reduce over the per-partition 25 rows
        -> [128,512] partials, then a single fp32 matmul by S -> psum.
    ACT copies psum->SBUF; outputs DMA out via SWDGE.
  - Host: concat per-core [625, 512] results, divide by counts, permute.
"""

import numpy as np

N_NODES = 500_000
PATH = 8
DIM = 64
N_GRAPHS = 5_000
COLS = PATH * DIM  # 512
N_CORES = 8
ROWS_PER_CORE = N_NODES // N_CORES      # 62500
GRAPHS_PER_CORE = N_GRAPHS // N_CORES   # 625
CNT = N_NODES // N_GRAPHS               # 100 rows per graph

R = 25                   # rows per partition
FULL_P = 128             # partitions in a full tile
TILE_ROWS = FULL_P * R   # 3200 rows per full tile
TILE_G = TILE_ROWS // CNT               # 32 graphs per full tile
N_FULL_TILES = 19
TAIL_ROWS = ROWS_PER_CORE - N_FULL_TILES * TILE_ROWS  # 1700
TAIL_P = TAIL_ROWS // R  # 68
TAIL_G = TAIL_ROWS // CNT               # 17
PE_EVERY = 3             # every 3rd full tile reduced on the PE (matmul) path
PS_BUFS = 4              # PSUM accumulator slots
OT_BUFS = 4              # output staging slots (must equal PS_BUFS: the
                         # absorber ldweights relies on ot[t-PS_BUFS] having
                         # been written by the ACT copy that freed ps slot t)

_CACHE = {}
LAST_RESULT = None  # BassKernelResults from the most recent run (for test.py)


def _build_nc(reps=1, dma_only=False):
    import concourse.bass as bass
    import concourse.mybir as mybir
    from concourse.tile import TileContext

    f32 = mybir.dt.float32
    nc = bass.Bass()

    x = nc.dram_tensor("x", [ROWS_PER_CORE, COLS], f32, kind="ExternalInput")
    y = nc.dram_tensor("y", [GRAPHS_PER_CORE, COLS], f32, kind="ExternalOutput")

    # 0/1 selection matrix: S[p, g] = 1 if partition p belongs to graph g.
    s_np = np.zeros((FULL_P, TILE_G), dtype=np.float32)
    for p in range(FULL_P):
        s_np[p, p // (CNT // R)] = 1.0
    s_dram = nc.inline_tensor(s_np, name="s_sel")

    n_dve_tiles = sum(
        1
        for t in range(N_FULL_TILES + 1)
        if t == N_FULL_TILES or t % PE_EVERY != 0
    )
    with TileContext(nc) as tc:
        with (
            tc.tile_pool(name="sconst", bufs=1) as spool,
            tc.tile_pool(name="xin", bufs=3) as xpool,
            tc.tile_pool(name="partials", bufs=n_dve_tiles) as ppool,
            tc.tile_pool(name="psum", bufs=PS_BUFS, space="PSUM") as cpool,
            tc.tile_pool(name="outbuf", bufs=OT_BUFS) as opool,
        ):
            s_sb = spool.tile([FULL_P, TILE_G], f32)
            nc.sync.dma_start(out=s_sb[:], in_=s_dram[:])

            # Sync-wait budget: each lowered ISA instruction carries very few
            # semaphore waits (the PE matmul/ldweights slots hold just one),
            # but Tile attaches every outstanding dependency to the first
            # instruction that touches a tile. fp32 matmuls are worst: they
            # self-load weights, so lhsT+rhs+psum deps all land on one
            # instruction. Keep every fp32 matmul at <=1 wait by absorbing
            # deps into bf16 LDWEIGHTS/matmul helpers (bf16 matmul emits a
            # separate LDWEIGHTS, spreading waits over two instructions; a
            # bare LDWEIGHTS has no Tile-tracked output, so it picks up only
            # the read dep we aim it at). The bf16 views are bitcasts of live
            # fp32 tiles — values are irrelevant, only the dependency counts.
            s_b16 = s_sb[:1, 0:1].bitcast(mybir.dt.bfloat16)  # [1, 2]
            nc.tensor.ldweights(s_b16)  # absorbs the S-matrix DMA wait

            ots = []
            for tg in range(reps * (N_FULL_TILES + 1)):
                t = tg % (N_FULL_TILES + 1)
                tail = t == N_FULL_TILES
                P = TAIL_P if tail else FULL_P
                G = TAIL_G if tail else TILE_G
                r0 = t * TILE_ROWS
                g0 = t * TILE_G

                xt = xpool.tile([FULL_P, R * COLS], f32, tag="xt")
                src = x[r0 : r0 + P * R, :].rearrange("(p r) c -> p (r c)", p=P)
                # Single issuing engine: alternating SP/ACT leaves each engine
                # with stale vector clocks and piles >2 sync waits onto DMA
                # instructions, which the PSEUDO_DMA ISA slot can't encode.
                nc.sync.dma_start(out=xt[:P, :], in_=src)
                if dma_only:
                    continue

                ps = cpool.tile([TILE_G, COLS], f32, tag="ps")
                if tg >= PS_BUFS:
                    # ot slot (t % OT_BUFS) was written by the same ACT copy
                    # that last read ps slot (t % PS_BUFS) — reading it syncs
                    # PE's view of ACT to exactly the needed tick.
                    nc.tensor.ldweights(ots[tg - PS_BUFS][:1, 0:1].bitcast(mybir.dt.bfloat16))
                    # PSUM WAW vs our own older accumulation group.
                    nc.tensor.matmul(
                        ps[:2, :2],
                        lhsT=s_b16,
                        rhs=s_b16,
                        start=True,
                        stop=True,
                        skip_group_check=True,
                    )
                if not tail and t % PE_EVERY == 0:
                    # PE path: accumulate 25 matmuls into PSUM.
                    for r in range(R):
                        nc.tensor.matmul(
                            ps[:G, :],
                            lhsT=s_sb[:P, :G],
                            rhs=xt[:P, r * COLS : (r + 1) * COLS],
                            start=(r == 0),
                            stop=(r == R - 1),
                        )
                else:
                    # DVE path: reduce 25 rows per partition, then one matmul.
                    # Single-use slots (bufs=N_DVE_TILES) so the reduce never
                    # carries WAW/read-release waits — only its input DMA.
                    pt = ppool.tile([FULL_P, COLS], f32, tag="pt")
                    nc.vector.tensor_reduce(
                        pt[:P, :],
                        xt[:P, :].rearrange("p (r c) -> p c r", r=R),
                        axis=mybir.AxisListType.X,
                        op=mybir.AluOpType.add,
                    )
                    nc.tensor.matmul(
                        ps[:G, :],
                        lhsT=s_sb[:P, :G],
                        rhs=pt[:P, :],
                        start=True,
                        stop=True,
                    )

                ot = opool.tile([TILE_G, COLS], f32, tag="ot")
                ots.append(ot)
                nc.scalar.copy(out=ot[:G, :], in_=ps[:G, :])
                nc.gpsimd.dma_start(out=y[g0 : g0 + G, :], in_=ot[:G, :])

    _split_excess_waits(nc)
    return nc


def _split_excess_waits(nc):
    """Walrus encodes at most one semaphore wait per compute/DMA instruction
    (setupSyncWait raises "Too many sync wait commands" otherwise), but Tile's
    semaphore assignment attaches every outstanding dependency to the first
    instruction touching a tile. Hoist all but the last wait of each
    multi-wait instruction into dedicated wait-only InstEventSemaphore
    instructions inserted just before it on the same engine — the sequencer
    blocks there instead, which is semantically identical.
    """
    import concourse.mybir as mybir

    skip = {
        "InstEventSemaphore",
        "InstCall",
        "InstUnconditionalBranch",
        "InstISA",
        "InstRegisterMove",
    }
    n_fix = 0
    for bb in nc.main_func.blocks:
        lst = bb.instructions
        i = 0
        while i < len(lst):
            ins = lst[i]
            si = ins.sync_info
            if (
                type(ins).__name__ not in skip
                and si is not None
                and len(si.on_wait) > 1
            ):
                waits = list(si.on_wait)
                for w in waits[:-1]:
                    ev = mybir.InstEventSemaphore(
                        name=f"W-split-{n_fix}", ins=[], outs=[]
                    )
                    n_fix += 1
                    ev.engine = ins.engine
                    ev.sync_info = mybir.SyncInfo(on_wait=[w], on_update=[])
                    lst.insert(i, ev)
                    i += 1
                ins.sync_info = mybir.SyncInfo(
                    on_wait=[waits[-1]], on_update=list(si.on_update)
                )
            i += 1
    return n_fix


def _numpy_fallback(features, counts):
    n = features.shape[0]
    g = counts.shape[0]
    x = np.transpose(features, (0, 2, 1)).reshape(n, -1)
    out = np.zeros((g, x.shape[1]), dtype=np.float32)
    idx = 0
    for i in range(g):
        c = int(counts[i])
        if c > 0:
            out[i] = x[idx : idx + c].sum(axis=0, dtype=np.float32)
        idx += c
    denom = np.maximum(counts, 1).astype(np.float32)[:, None]
    return (out / denom).astype(np.float32)


def kernel(features, counts, _trace=False, _trace_cores=None):
    global LAST_RESULT
    features = np.ascontiguousarray(np.asarray(features, dtype=np.float32))
    counts = np.asarray(counts, dtype=np.int32)

    if (
        features.shape != (N_NODES, PATH, DIM)
        or counts.shape != (N_GRAPHS,)
        or not np.all(counts == CNT)
    ):
        return _numpy_fallback(features, counts)

    from concourse.bass_utils import run_bass_kernel_spmd

    if "nc" not in _CACHE:
        _CACHE["nc"] = _build_nc()
    nc = _CACHE["nc"]

    x_flat = features.reshape(N_NODES, COLS)
    in_maps = [
        {"x": x_flat[c * ROWS_PER_CORE : (c + 1) * ROWS_PER_CORE]}
        for c in range(N_CORES)
    ]

    res = run_bass_kernel_spmd(
        nc,
        in_maps,
        core_ids=list(range(N_CORES)),
        trace=_trace,
        trace_cores=_trace_cores,
    )
    LAST_RESULT = res

    sums = np.concatenate([r["y"] for r in res.results], axis=0)  # [5000, 512] (p,d)
    denom = np.maximum(counts, 1).astype(np.float32)[:, None]
    means = sums / denom
    # (g, p, d) -> (g, d, p) to match reference layout
    out = means.reshape(N_GRAPHS, PATH, DIM).transpose(0, 2, 1).reshape(N_GRAPHS, COLS)
    return np.ascontiguousarray(out.astype(np.float32))
